# revision 2
# baseline (speedup 1.0000x reference)
"""3-layer GCN on 8 TRN2 cores — v2 (f32 gathers, restructured pipeline).

vs v1 baseline:
- S-matrices built batched (one f32 tensor_tensor is_equal per gather call
  with a stride-0 broadcast of the per-column dstl scalar) instead of one
  tensor_scalar per column.
- PSUM accumulation per 128-dst block across the cats of a src half-table;
  16 block accumulators packed per PSUM bank ([128,512] f32 bank tiles,
  matmuls target 32-wide slices). Seeded by an identity matmul with the
  self-loop term (round A) / the round-A partial (round B): no SBUF
  accumulator adds, self loops excluded from the edge list.
- Table split in two halves (= all cores' local half-chunks); the
  AllGather is per-half and overlaps with compute.
- Next-layer x@W fused into the epilogue (transpose+matmul per block).
- Half-column (64-edge) padding quantum with same-cell merging.
Gathers stay f32/128B blocking dma_gather (HW-proven geometry).
"""

import os
import textwrap
import inspect
import numpy as np

import concourse.bass as bass
import concourse.bacc as bacc
import concourse.mybir as mybir
import concourse.tile as tile
from concourse import bass_utils
from concourse.masks import make_identity

F32 = mybir.dt.float32
I16 = mybir.dt.int16

C = 8
P = 128
F_IN = 128
H = 32
CELL = 64
WPAIRS = 32768          # pairs addressable by int16 idx (256B stride)

N = int(os.environ.get("GCN_N", "200000"))
NPC = N // C
HPC = NPC // 2
HPAD = ((HPC + 44 + P - 1) // P) * P     # >=44 pad rows, 128-aligned
NPAD = 2 * HPAD
NCH = NPAD // P
HTBL = C * HPAD
NWIN = (HTBL // 2 + WPAIRS - 1) // WPAIRS    # pair windows per half-table
GB = 32                 # blocks per psum group (2 banks of 16)
NG = (NCH + GB - 1) // GB
TCOLS = 32              # max columns per gather call
NCAT = 2 * NWIN * 2     # (half, win, parity)


def _patch_dma_gather():
    if getattr(bass.BassGpSimd.dma_gather, "_relaxed", False):
        return
    src = textwrap.dedent(inspect.getsource(bass.BassGpSimd.dma_gather))
    assert "elem_size_bytes % 256 == 0" in src
    src = src.replace("elem_size_bytes % 256 == 0", "elem_size_bytes % 64 == 0")
    ns = {}
    exec(compile(src, "<dma_gather_patched>", "exec"), vars(bass).copy(), ns)
    fn = ns["dma_gather"]
    fn._relaxed = True
    bass.BassGpSimd.dma_gather = fn


_patch_dma_gather()


# ---------------------------------------------------------------- host prep
def _prepare(edge_index):
    src = np.asarray(edge_index[0], dtype=np.int64)
    dst = np.asarray(edge_index[1], dtype=np.int64)
    E = len(src)

    deg = np.bincount(dst, minlength=N).astype(np.float64) + 1.0  # + self loop
    dinv = (1.0 / np.sqrt(deg)).astype(np.float32)

    def to_slot(v):
        c = v // NPC
        l = v - c * NPC
        h = (l >= HPC).astype(np.int64)
        return c, h * HPAD + (l - h * HPC)

    c_s, slot_s = to_slot(src)
    gpos_s = (slot_s // HPAD) * HTBL + c_s * HPAD + (slot_s % HPAD)
    c_d, slot_d = to_slot(dst)

    hh = gpos_s // HTBL                      # src half (0/1)
    hloc = gpos_s % HTBL
    wi = (hloc // 2) // WPAIRS               # window within half
    par = gpos_s % 2
    cat = hh * (2 * NWIN) + wi * 2 + par
    pidx = (hloc // 2) - wi * WPAIRS
    assert pidx.max() < WPAIRS and pidx.min() >= 0
    blk = slot_d // P
    NBLK = NPAD // P

    flat = (c_d * NCAT + cat) * NBLK + blk
    counts = np.bincount(flat, minlength=C * NCAT * NBLK).reshape(
        C, NCAT, NBLK)
    nh = (-(-counts // CELL)).max(axis=0)    # 64-halfslots per (cat, block)

    # zero row (pair idx, window-local) for each (half, win, parity)
    zpair = np.full((2, NWIN, 2), -1, np.int64)
    for c in range(C):
        for h in range(2):
            for r in range(HPC, HPAD):
                gp = h * HTBL + c * HPAD + r
                hl = gp % HTBL
                zw = (hl // 2) // WPAIRS
                zpair[h, zw, gp % 2] = (hl // 2) - zw * WPAIRS
    used_wins = set((int(k) // 2) % (2 * NWIN) // 2 * 0 for k in [])  # unused

    # ---------------- column schedule ----------------
    # a column = 128 edges of ONE (cat, 128-dst-block): two 64-halfslots of
    # the same block paired; every matmul is full-K at partition base 0.
    col_blk = []
    calls = []
    hs_index = {}
    pos = 0
    for wdw in range(2):
        for g in range(NG):
            blocks_g = range(g * GB, min((g + 1) * GB, NBLK))
            for k in range(wdw * 2 * NWIN, (wdw + 1) * 2 * NWIN):
                c0 = pos
                for bi in blocks_g:
                    nhh = int(nh[k, bi])
                    for jc in range((nhh + 1) // 2):
                        hs_index[(k, bi, 2 * jc)] = (pos, 0)
                        if 2 * jc + 1 < nhh:
                            hs_index[(k, bi, 2 * jc + 1)] = (pos, 64)
                        col_blk.append(bi)
                        pos += 1
                cc = c0
                while cc < pos:
                    n = min(TCOLS, pos - cc)
                    calls.append((k, cc, n))
                    cc += n
    TOTCOLS = pos

    # ---------------- slot filling ----------------
    idx_all = np.empty((C, TOTCOLS * P), np.int16)
    dstl_all = np.full((C, TOTCOLS * P), float(P), np.float32)
    # per-column pad idx = zero row of its cat
    colcat = np.empty(TOTCOLS, np.int64)
    for (k, c0, ncols) in calls:
        colcat[c0:c0 + ncols] = k
    for k in range(NCAT):
        h, r = divmod(k, 2 * NWIN)
        zw, zp = divmod(r, 2)
        z = zpair[h, zw, zp]
        if (colcat == k).any():
            assert z >= 0, f"no zero row for cat {k}"
        m = np.repeat(colcat == k, P)
        idx_all[:, m] = z

    order = np.lexsort((pidx, blk, cat, c_d))
    oc, ok, oci = c_d[order], cat[order], blk[order]
    oq, ooff = pidx[order], (slot_d[order] % P)
    key = (oc * NCAT + ok) * NBLK + oci
    first = np.r_[True, key[1:] != key[:-1]]
    run_start = np.flatnonzero(first)
    run_id = np.cumsum(first) - 1
    within = np.arange(E) - run_start[run_id]
    hsj = within // CELL
    win64 = within % CELL
    max_nh = int(nh.max())
    lut_col = np.full((NCAT, NBLK, max_nh), -1, np.int64)
    lut_row = np.zeros((NCAT, NBLK, max_nh), np.int64)
    for (k, ci, j), (col, row0) in hs_index.items():
        lut_col[k, ci, j] = col
        lut_row[k, ci, j] = row0
    colarr = lut_col[ok, oci, hsj]
    row0arr = lut_row[ok, oci, hsj]
    assert (colarr >= 0).all()
    slotpos = colarr * P + row0arr + win64
    idx_all[oc, slotpos] = oq.astype(np.int16)
    dstl_all[oc, slotpos] = ooff

    return dict(
        dinv=dinv, TOTCOLS=TOTCOLS, calls=calls, col_blk=col_blk,
        idx_all=idx_all, dstl_all=dstl_all,
    )


# ---------------------------------------------------------------- bass build
def _build(plan):
    TOTCOLS = plan["TOTCOLS"]
    calls = plan["calls"]
    col_blk = plan["col_blk"]

    nc = bacc.Bacc("TRN2", target_bir_lowering=False, debug=False,
                   num_devices=C)

    xT_t = nc.dram_tensor("xT", [F_IN, NPAD], F32, kind="ExternalInput")
    idx_t = nc.dram_tensor("idx", [P, TOTCOLS * 8], I16, kind="ExternalInput")
    dstl_t = nc.dram_tensor("dstl", [P, TOTCOLS], F32, kind="ExternalInput")
    dinv_t = nc.dram_tensor("dinv", [P, NCH], F32, kind="ExternalInput")
    iota_t = nc.dram_tensor("iota", [P, TCOLS * P], F32,
                            kind="ExternalInput")
    W1_t = nc.dram_tensor("W1", [F_IN, H], F32, kind="ExternalInput")
    W2_t = nc.dram_tensor("W2", [H, H], F32, kind="ExternalInput")
    W3_t = nc.dram_tensor("W3", [H, H], F32, kind="ExternalInput")
    Wl_t = nc.dram_tensor("Wl", [H, F_IN], F32, kind="ExternalInput")
    brep_t = nc.dram_tensor("brep", [P, 3 * H], F32, kind="ExternalInput")
    blin_t = nc.dram_tensor("blin", [P, F_IN], F32, kind="ExternalInput")
    out_t = nc.dram_tensor("out", [NPAD, F_IN], F32, kind="ExternalOutput")

    agins, tabs = [], []
    for L in range(3):
        agins.append((nc.dram_tensor(f"aga{L}", [HPAD, H], F32),
                      nc.dram_tensor(f"agb{L}", [HPAD, H], F32)))
        tabs.append((nc.dram_tensor(f"taba{L}", [HTBL, H], F32),
                     nc.dram_tensor(f"tabb{L}", [HTBL, H], F32)))

    Sig = mybir.ActivationFunctionType.Sigmoid
    ISEQ = mybir.AluOpType.is_equal
    rg = [list(range(C))]

    nmm = np.zeros((2, NG, GB), np.int64)
    call_meta = []
    for (k, c0, ncols) in calls:
        wdw = k // (2 * NWIN)
        metas = []
        for j in range(ncols):
            bi = col_blk[c0 + j]
            g = bi // GB
            jb = bi - g * GB
            nmm[wdw, g, jb] += 1
            metas.append((j, jb))
        call_meta.append(metas)

    def blocks_of(g):
        return range(g * GB, min((g + 1) * GB, NCH))

    with tile.TileContext(nc) as tc:
        with (
            tc.tile_pool(name="cst", bufs=1) as cst,
            tc.tile_pool(name="sb", bufs=3) as sb,
            tc.tile_pool(name="gth", bufs=3) as gp,
            tc.tile_pool(name="mmp", bufs=2, space="PSUM") as mp,
            tc.tile_pool(name="blkp", bufs=2, space="PSUM") as bp,
        ):
            ident = cst.tile([P, P], F32)
            make_identity(nc, ident[:])
            zt = cst.tile([P, P], F32)
            nc.vector.memset(zt[:], 0.0)
            w1 = cst.tile([F_IN, H], F32)
            nc.sync.dma_start(out=w1[:], in_=W1_t.ap())
            w2 = cst.tile([H, H], F32)
            nc.sync.dma_start(out=w2[:], in_=W2_t.ap())
            w3 = cst.tile([H, H], F32)
            nc.sync.dma_start(out=w3[:], in_=W3_t.ap())
            wl = cst.tile([H, F_IN], F32)
            nc.sync.dma_start(out=wl[:], in_=Wl_t.ap())
            brep = cst.tile([P, 3 * H], F32)
            nc.sync.dma_start(out=brep[:], in_=brep_t.ap())
            blin = cst.tile([P, F_IN], F32)
            nc.sync.dma_start(out=blin[:], in_=blin_t.ap())
            dinv_sb = cst.tile([P, NCH], F32)
            nc.sync.dma_start(out=dinv_sb[:], in_=dinv_t.ap())
            iota = cst.tile([P, TCOLS * P], F32)
            nc.sync.dma_start(out=iota[:], in_=iota_t.ap())
            acc = cst.tile([P, NCH * H], F32)
            tbo0 = cst.tile([P, NCH * H], F32)
            tbo1 = cst.tile([P, NCH * H], F32)
            tbl_own = [tbo0, tbo1]

            LAYER = [0]

            def do_gather(k, c0, ncols):
                hh, r = divmod(k, 2 * NWIN)
                wi, par = divmod(r, 2)
                it = sb.tile([P, TCOLS * 8], I16, tag="idx")
                nc.sync.dma_start(
                    out=it[:, :ncols * 8],
                    in_=idx_t.ap()[:, c0 * 8:(c0 + ncols) * 8])
                dl = sb.tile([P, TCOLS], F32, tag="dl")
                nc.sync.dma_start(
                    out=dl[:, :ncols], in_=dstl_t.ap()[:, c0:c0 + ncols])
                pv = tabs[LAYER[0]][hh].ap().rearrange(
                    "(q two) f -> q (two f)", two=2)
                rows = min(WPAIRS, HTBL // 2 - wi * WPAIRS)
                in_ap = pv[wi * WPAIRS:wi * WPAIRS + rows,
                           par * H:(par + 1) * H]
                g = gp.tile([P, TCOLS * H], F32, tag="g")
                nc.gpsimd.dma_gather(
                    out_ap=g[:, :ncols * H].rearrange("p (c e) -> p c e", e=H),
                    in_ap=in_ap,
                    idxs_ap=it[:, :ncols * 8],
                    num_idxs=ncols * P, num_idxs_reg=ncols * P,
                    elem_size=H, elem_step=2 * H,
                    single_packet=False,
                )
                return g, dl

            def fused_next(b, h_t, L):
                tp = mp.tile([H, P], F32, tag="tp")
                nc.tensor.transpose(out=tp[:], in_=h_t[:], identity=ident[:])
                hT = sb.tile([H, P], F32, tag="hT")
                nc.scalar.copy(out=hT[:], in_=tp[:])
                if L < 2:
                    wnext = (w2, w3)[L]
                    am = mp.tile([P, F_IN], F32, tag="mo")
                    nc.tensor.matmul(out=am[:, :H], lhsT=hT[:], rhs=wnext[:],
                                     start=True, stop=True)
                    tslice = tbl_own[(L + 1) % 2][:, b * H:(b + 1) * H]
                    nc.vector.tensor_scalar_mul(
                        tslice, am[:, :H], dinv_sb[:, b:b + 1])
                    half, lb = (0, b) if b < NCH // 2 else (1, b - NCH // 2)
                    nc.sync.dma_start(
                        out=agins[L + 1][half].ap()[lb * P:(lb + 1) * P, :],
                        in_=tslice)
                else:
                    pf = mp.tile([P, F_IN], F32, tag="mo")
                    nc.tensor.matmul(out=pf[:], lhsT=hT[:], rhs=wl[:],
                                     start=True, stop=True)
                    of = sb.tile([P, F_IN], F32, tag="of")
                    nc.vector.tensor_add(of[:], pf[:], blin[:])
                    o2 = sb.tile([P, F_IN], F32, tag="o2")
                    nc.scalar.activation(o2[:], of[:], Sig)
                    nc.sync.dma_start(
                        out=out_t.ap()[b * P:(b + 1) * P, :], in_=o2[:])

            def fire_ag(L, half):
                nc.gpsimd.collective_compute(
                    "AllGather", mybir.AluOpType.bypass, replica_groups=rg,
                    ins=[agins[L][half].ap().opt()],
                    outs=[tabs[L][half].ap().opt()],
                )

            # ---------------- layer 0 mm ----------------
            for i in range(NCH):
                lhsT = sb.tile([F_IN, P], F32, tag="xt")
                nc.sync.dma_start(
                    out=lhsT[:], in_=xT_t.ap()[:, i * P:(i + 1) * P])
                pt0 = mp.tile([P, F_IN], F32, tag="mo")
                nc.tensor.matmul(out=pt0[:, :H], lhsT=lhsT[:], rhs=w1[:],
                                 start=True, stop=True)
                tslice = tbl_own[0][:, i * H:(i + 1) * H]
                nc.vector.tensor_scalar_mul(tslice, pt0[:, :H],
                                            dinv_sb[:, i:i + 1])
                half, lb = (0, i) if i < NCH // 2 else (1, i - NCH // 2)
                nc.sync.dma_start(
                    out=agins[0][half].ap()[lb * P:(lb + 1) * P, :],
                    in_=tslice)
            fire_ag(0, 0)
            fire_ag(0, 1)

            # ---------------- prop layers ----------------
            for L in range(3):
                LAYER[0] = L
                ci_ptr = 0
                for wdw in range(2):
                    for g in range(NG):
                        nb = len(list(blocks_of(g)))
                        ptA = bp.tile([P, 512], F32, tag="bnkA")
                        ptB = bp.tile([P, 512], F32, tag="bnkB",
                                      name="ptB") if nb > 16 else None

                        def pt_of(jb):
                            if jb < 16:
                                return ptA[:, jb * H:(jb + 1) * H]
                            return ptB[:, (jb - 16) * H:(jb - 15) * H]

                        # one accumulation chain per psum bank: first
                        # matmul start=True zeroes the whole 2KB region; a
                        # closing full-bank zero matmul gets stop=True so
                        # every epilogue read depends on it.
                        bank_done = np.zeros(2, np.int64)

                        def bflags(jb):
                            bk = jb // 16
                            bank_done[bk] += 1
                            return (bank_done[bk] == 1, False)

                        for jb in range(nb):
                            b = g * GB + jb
                            st, sp = bflags(jb)
                            if wdw == 0:
                                nc.tensor.matmul(
                                    out=pt_of(jb), lhsT=ident[:],
                                    rhs=tbl_own[L % 2][:, b * H:(b + 1) * H],
                                    start=st, stop=sp)
                            else:
                                nc.tensor.matmul(
                                    out=pt_of(jb), lhsT=ident[:],
                                    rhs=acc[:, b * H:(b + 1) * H],
                                    start=st, stop=sp)
                        for k in range(wdw * 2 * NWIN, (wdw + 1) * 2 * NWIN):
                            while ci_ptr < len(calls) and \
                                    calls[ci_ptr][0] == k and \
                                    col_blk[calls[ci_ptr][1]] // GB == g:
                                kk, c0, ncols = calls[ci_ptr]
                                if os.environ.get("GCN_V2_NOCALLS") == "1":
                                    ci_ptr += 1
                                    continue
                                gt, dl = do_gather(k, c0, ncols)
                                S = sb.tile([P, TCOLS * P], F32, tag="S",
                                            bufs=2)
                                nc.vector.tensor_tensor(
                                    out=S[:, :ncols * P].rearrange(
                                        "p (c d) -> p c d", d=P),
                                    in0=iota[:, :ncols * P].rearrange(
                                        "p (c d) -> p c d", d=P),
                                    in1=dl[:, :ncols].unsqueeze(2)
                                        .broadcast_to([P, ncols, P]),
                                    op=ISEQ,
                                )
                                for (j, jb) in call_meta[ci_ptr]:
                                    st, sp = bflags(jb)
                                    nc.tensor.matmul(
                                        out=pt_of(jb),
                                        lhsT=S[:, j * P:(j + 1) * P],
                                        rhs=gt[:, j * H:(j + 1) * H],
                                        start=st, stop=sp,
                                    )
                                ci_ptr += 1
                        wA = min(nb, 16) * H
                        nc.tensor.matmul(
                            out=ptA[:, :wA], lhsT=zt[:], rhs=iota[:, :wA],
                            start=False, stop=True)
                        if ptB is not None:
                            wB = (nb - 16) * H
                            nc.tensor.matmul(
                                out=ptB[:, :wB], lhsT=zt[:], rhs=iota[:, :wB],
                                start=False, stop=True)
                        for jb in range(nb):
                            b = g * GB + jb
                            if wdw == 0:
                                nc.scalar.copy(
                                    out=acc[:, b * H:(b + 1) * H],
                                    in_=pt_of(jb))
                            else:
                                t1 = sb.tile([P, H], F32, tag="t1")
                                nc.vector.tensor_scalar_mul(
                                    t1[:], pt_of(jb), dinv_sb[:, b:b + 1])
                                t2 = sb.tile([P, H], F32, tag="t2")
                                nc.vector.tensor_add(
                                    t2[:], t1[:], brep[:, L * H:(L + 1) * H])
                                if L < 2:
                                    ht = sb.tile([P, H], F32, tag="ht")
                                    nc.scalar.activation(ht[:], t2[:], Sig)
                                else:
                                    ht = t2
                                fused_next(b, ht, L)
                        if wdw == 1 and L < 2 and g == NG - 1:
                            fire_ag(L + 1, 0)
                            fire_ag(L + 1, 1)

    nc.compile()
    return nc


# ---------------------------------------------------------------- entry
_CACHE = {}


def kernel(x, edge_index, W1, b1, W2, b2, W3, b3, Wlin, blin):
    x = np.asarray(x, dtype=np.float32)
    edge_index = np.asarray(edge_index)
    W1 = np.asarray(W1, dtype=np.float32)
    W2 = np.asarray(W2, dtype=np.float32)
    W3 = np.asarray(W3, dtype=np.float32)
    Wlin = np.asarray(Wlin, dtype=np.float32)
    b1 = np.asarray(b1, dtype=np.float32)
    b2 = np.asarray(b2, dtype=np.float32)
    b3 = np.asarray(b3, dtype=np.float32)
    blin = np.asarray(blin, dtype=np.float32)

    plan = _prepare(edge_index)

    key = (x.shape[0], edge_index.shape[1], plan["TOTCOLS"])
    if key not in _CACHE:
        _CACHE[key] = _build(plan)
    nc = _CACHE[key]

    dinv = plan["dinv"]
    brep = np.concatenate([
        np.tile(b1[None, :], (P, 1)),
        np.tile(b2[None, :], (P, 1)),
        np.tile(b3[None, :], (P, 1)),
    ], axis=1).astype(np.float32)
    blin_rep = np.tile(blin[None, :], (P, 1)).astype(np.float32)
    iota = np.tile(np.arange(P, dtype=np.float32)[None, :], (P, TCOLS))

    v = np.arange(N, dtype=np.int64)
    vc = v // NPC
    vl = v % NPC
    vh = (vl >= HPC).astype(np.int64)
    vslot = vh * HPAD + (vl - vh * HPC)

    in_maps = []
    for c in range(C):
        mask = vc == c
        xT = np.zeros((F_IN, NPAD), dtype=np.float32)
        xT[:, vslot[mask]] = x[mask].T
        dinv_pad = np.zeros(NPAD, np.float32)
        dinv_pad[vslot[mask]] = dinv[mask]
        idxc = plan["idx_all"][c].reshape(plan["TOTCOLS"] * 8, 16).T
        idxc = np.tile(idxc, (8, 1))
        dstlc = plan["dstl_all"][c].reshape(plan["TOTCOLS"], P).T
        in_maps.append({
            "xT": xT,
            "idx": np.ascontiguousarray(idxc),
            "dstl": np.ascontiguousarray(dstlc),
            "dinv": np.ascontiguousarray(dinv_pad.reshape(NCH, P).T),
            "iota": iota,
            "W1": W1, "W2": W2, "W3": W3, "Wl": Wlin,
            "brep": brep, "blin": blin_rep,
        })

    mode = os.environ.get("GCN_BASS_MODE", "hw")
    if mode == "sim":
        from concourse.bass_interp import MultiCoreSim
        sim = MultiCoreSim(nc, C)
        for c in range(C):
            for name, arr in in_maps[c].items():
                sim.cores[c].tensor(name)[:] = arr
        sim.simulate(check_with_hw=False)
        outs = [np.array(sim.cores[c].mem_tensor("out")) for c in range(C)]
    else:
        res = bass_utils.run_bass_kernel_spmd(nc, in_maps, list(range(C)))
        outs = [res.results[c]["out"] for c in range(C)]

    full = np.empty((N, F_IN), dtype=np.float32)
    for c in range(C):
        m = vc == c
        full[np.flatnonzero(m)] = outs[c][vslot[m]]
    return full
